# revision 84
# baseline (speedup 1.0000x reference)
"""LoFTR encoder layer (linear attention) on 8 Trainium2 NeuronCores.

Sharding: core c -> (n = c//2, L-half = c%2). Each core processes 4096 query
tokens; K/V state (KV = K^T V' over full S=8192) is computed replicated per
batch element. All matmuls bf16 (fp32 PSUM accumulate). Activations flow
feature-major; both LayerNorms run token-major (per-partition stats).

Pipeline structure: 256-token work units, multi-buffered PSUM pools
(8 banks exactly), phase 2 emitted as a skewed 5-stage software pipeline
so each in-order engine queue matches readiness order. KV state is
reassociated as (elu(K)^T [src|1]) @ Wv^T so no V projection or V copy
runs per source group. elu(x)+1 = max(x,0) + min(exp(x),1) (exp straight
from PSUM). Z = 1/(Q.Ksum) via a custom-DVE reciprocal on a
head-replicated qk matmul (no DRAM round trip; eps negligible against
qk ~ 1e5). One activation table set (ln/exp/copy/relu) loaded once.
Inputs stream in chunks over the SP-engine HWDGE queues.
"""

import numpy as np
import ml_dtypes

import concourse.bass as bass
import concourse.bacc as bacc
import concourse.tile as tile
from concourse import mybir
from concourse.bass_utils import run_bass_kernel_spmd

F32 = mybir.dt.float32
BF16 = mybir.dt.bfloat16
FP8 = mybir.dt.float8e4
AF = mybir.ActivationFunctionType
ALU = mybir.AluOpType

D_MODEL = 256
NHEAD = 8
HEAD_DIM = 32
LN_EPS = 1e-7
ATTN_EPS = 1e-6

ACT_SET_LN_EXP = 6   # act_info.json set with ln+exp+copy+relu: one table load


def build_kernel(nc, TBLK, SBLK, g2_is_one=True, b1p_is_zero=True):
    """Emit the per-core program. TBLK = query-token 128-blocks (32 full),
    SBLK = source-token 128-blocks (64 full). Returns nothing; declares
    DRAM tensors by name."""
    C = D_MODEL
    NT = TBLK // 2    # q-units of 256 tokens (16)
    NSG = SBLK // 2   # source groups of 256 rows (32)

    # ---- DRAM I/O (per-core, host pre-shaped) ----
    x_pre = nc.dram_tensor("x_pre", [128, 2, TBLK * 128], BF16,
                           kind="ExternalInput").ap()
    s_pre = nc.dram_tensor("s_pre", [128, 2, SBLK * 128], BF16,
                           kind="ExternalInput").ap()
    s_tok = nc.dram_tensor("s_tok", [128, SBLK * 264], BF16,
                           kind="ExternalInput").ap()
    wq_t = nc.dram_tensor("wq_t", [128, 2, 2, 128], FP8, kind="ExternalInput").ap()
    xq_d = nc.dram_tensor("x_q", [128, 2, TBLK * 128], FP8,
                          kind="ExternalInput").ap()
    wk_r = nc.dram_tensor("wk_r", [128, 2, 256], BF16, kind="ExternalInput").ap()
    wv_r = nc.dram_tensor("wv_r", [128, 2, 256], BF16, kind="ExternalInput").ap()
    wm_r = nc.dram_tensor("wm_r", [128, 2, 256], BF16, kind="ExternalInput").ap()
    w1_t = nc.dram_tensor("w1_t", [128, 4, 4, 128], BF16, kind="ExternalInput").ap()
    w2_r = nc.dram_tensor("w2_r", [128, 4, 256], BF16, kind="ExternalInput").ap()
    b1p_d = nc.dram_tensor("b1p", [128, 4], F32, kind="ExternalInput").ap()
    g2rep_d = nc.dram_tensor("g2rep", [128, 256], F32, kind="ExternalInput").ap()
    ident_d = nc.dram_tensor("ident", [128, 128], BF16, kind="ExternalInput").ap()
    maskbd_d = nc.dram_tensor("maskbd", [128, 128], BF16, kind="ExternalInput").ap()
    res_d = nc.dram_tensor("res", [128, TBLK * C], F32, kind="ExternalOutput").ap()

    from contextlib import ExitStack
    tc = nc.tc  # TileContext stored by caller
    ctx = ExitStack()
    nc._pool_ctx = ctx

    consts = ctx.enter_context(tc.tile_pool(name="consts", bufs=1))
    persist = ctx.enter_context(tc.tile_pool(name="persist", bufs=1))
    work = ctx.enter_context(tc.tile_pool(name="work", bufs=2))
    psA_cm = tc.tile_pool(name="psA", bufs=1, space="PSUM")
    psA = psA_cm.__enter__()

    # ---- constants ----
    wq = consts.tile([128, 2, 2, 128], FP8, name="wq")
    wk = consts.tile([128, 2, 256], BF16, name="wk")
    wv = consts.tile([128, 2, 256], BF16, name="wv")
    wm = consts.tile([128, 2, 256], BF16, name="wm")
    w1 = consts.tile([128, 4, 4, 128], BF16, name="w1")
    w2 = consts.tile([128, 4, 256], BF16, name="w2")
    b1p = consts.tile([128, 4], F32, name="b1p")
    g2rep = consts.tile([128, 256], F32, name="g2rep")
    eps_l = consts.tile([128, 1], F32, name="eps_l")
    ident = consts.tile([128, 128], BF16, name="ident")
    maskbd = consts.tile([128, 128], BF16, name="maskbd")

    # ---- persistent activations ----
    xf = persist.tile([128, 2, TBLK * 128], BF16, name="xf")      # x feature-major
    xq = persist.tile([128, 2, TBLK * 128], FP8, name="xq")       # x fp8 for Q
    qe = persist.tile([128, 2, TBLK * 128], BF16, name="qe")      # elu(q)+1 fm
    srcf = persist.tile([128, 2, SBLK * 128], BF16, name="srcf")  # source fm
    srct = persist.tile([128, SBLK * 264], BF16, name="srct")     # source tok-major

    for dst, src in ((wq, wq_t), (wk, wk_r), (wv, wv_r), (wm, wm_r),
                     (w1, w1_t), (w2, w2_r), (b1p, b1p_d), (g2rep, g2rep_d),
                     (ident, ident_d), (maskbd, maskbd_d)):
        nc.sync.dma_start(out=dst[:], in_=src)
    nc.vector.memset(eps_l, LN_EPS)

    # inputs arrive from host pre-shaped; chunked so compute starts as soon
    # as the first source rows land (HWDGE queues via the idle SP engine)
    XCH = TBLK * 128 // 2
    nc.sync.dma_start(out=xq[:, :, 0:XCH], in_=xq_d[:, :, 0:XCH])
    nc.sync.dma_start(out=xf[:, :, 0:XCH], in_=x_pre[:, :, 0:XCH])
    fr = [(0, 8), (8, 16), (16, 32), (32, 48), (48, 64)]  # 128-blocks
    for a, b in fr:
        nc.sync.dma_start(out=srcf[:, :, a * 128:b * 128],
                          in_=s_pre[:, :, a * 128:b * 128])
        nc.sync.dma_start(out=srct[:, a * 264:b * 264],
                          in_=s_tok[:, a * 264:b * 264])
    nc.sync.dma_start(out=xf[:, :, XCH:2 * XCH], in_=x_pre[:, :, XCH:2 * XCH])
    nc.sync.dma_start(out=xq[:, :, XCH:2 * XCH], in_=xq_d[:, :, XCH:2 * XCH])

    # T1 = elu(K)^T @ [src | 1] accumulated over all S in PSUM; KV is then
    # T1[:, :256] @ Wv^T (reassociated V: no V projection or V copy per
    # group) and Ksum is column 256.
    # halves padded to 512 so each accumulation target stays inside one
    # PSUM bank (a matmul output cannot cross a bank boundary)
    t1_ps = psA.tile([128, 2, 512], F32, name="t1_ps")

    def emit_src_group(g):
        """K projection + elu + T1 accumulation for 256 source rows."""
        k_ps = psA.tile([128, 2, 256], F32, name="k_ps", bufs=4)
        for j in range(2):
            scols = slice(256 * g + 128 * j, 256 * g + 128 * (j + 1))
            for k in range(2):
                nc.tensor.matmul(k_ps[:, j, :], lhsT=srcf[:, k, scols],
                                 rhs=wk[:, k, :], start=(k == 0), stop=(k == 1))
        # elu(k)+1 = max(k,0) + min(exp(k),1)
        ek = work.tile([128, 2, 256], BF16, name="ek", bufs=4)
        nc.scalar.activation(out=ek[:], in_=k_ps[:], func=AF.Exp, scale=1.0)
        mek = work.tile([128, 2, 256], BF16, name="mek", bufs=4)
        eng = nc.gpsimd if g % 2 == 0 else nc.vector
        eng.tensor_scalar_min(mek[:], ek[:], 1.0)
        ke = work.tile([128, 2, 256], BF16, name="ke", bufs=4)
        nc.vector.scalar_tensor_tensor(
            out=ke[:], in0=k_ps[:], scalar=0.0, in1=mek[:],
            op0=ALU.max, op1=ALU.add)
        for j in range(2):
            for c in range(2):
                nc.tensor.matmul(
                    t1_ps[:, c, 0:257],
                    lhsT=ke[:, j, 128 * c:128 * (c + 1)],
                    rhs=srct[:, 264 * (2 * g + j):264 * (2 * g + j) + 257],
                    start=(g == 0 and j == 0), stop=(g == NSG - 1 and j == 1))

    def emit_q_tile(t):
        """Q projection + elu for 256 query tokens."""
        cols = slice(256 * t, 256 * (t + 1))
        q_ps = psA.tile([128, 2, 256], F32, name="q_ps", bufs=2)
        for m in range(2):
            nc.tensor.matmul(q_ps[:, m, :], lhsT=wq[:, :, m, :],
                             rhs=xq[:, :, cols], start=True, stop=True,
                             perf_mode=mybir.MatmulPerfMode.DoubleRow)
        eq = work.tile([128, 2, 256], BF16, name="eq", bufs=4)
        nc.scalar.activation(out=eq[:], in_=q_ps[:], func=AF.Exp, scale=1.0)
        meq = work.tile([128, 2, 256], BF16, name="meq", bufs=4)
        nc.gpsimd.tensor_scalar_min(meq[:], eq[:], 1.0)
        nc.vector.scalar_tensor_tensor(
            out=qe[:, :, cols], in0=q_ps[:], scalar=0.0, in1=meq[:],
            op0=ALU.max, op1=ALU.add)

    # interleave source groups (T1/KV is the long pole) with q tiles;
    # two q tiles up front give the PE work while source rows stream in
    emit_q_tile(0)
    emit_q_tile(1)
    for g in range(NSG):
        emit_src_group(g)
        if g % 2 == 1 and g // 2 + 2 < NT:
            emit_q_tile(g // 2 + 2)

    # ---- tail: KV = T1 @ Wv^T, then BD (block-diag KV) + replicated Ksum ----
    ksumrep = consts.tile([128, 2, 128], BF16, name="ksumrep")
    for c in range(2):
        # ksumrep[f, c, f'] = Ksum[f] if head(f)==head(f') else 0, so
        # lhsT=ksumrep gives qk replicated over each head's 32 rows.
        nc.vector.tensor_scalar_mul(ksumrep[:, c, :], maskbd[:],
                                    t1_ps[:, c, 256:257])
    t1c = work.tile([128, 2, 256], BF16, name="t1c")
    nc.scalar.activation(out=t1c[:], in_=t1_ps[:, :, 0:256], func=AF.Copy)
    t1T_ps = psA.tile([128, 2, 256], BF16, name="t1T_ps", tag="k_ps", bufs=4)
    for c in range(2):
        for ci in range(2):
            nc.tensor.transpose(out=t1T_ps[:, ci, 128 * c:128 * (c + 1)],
                                in_=t1c[:, c, 128 * ci:128 * (ci + 1)],
                                identity=ident[:])
    t1T = work.tile([128, 2, 256], BF16, name="t1T")
    nc.vector.tensor_copy(out=t1T[:], in_=t1T_ps[:])
    kv_ps = psA.tile([128, 2, 256], F32, name="kv_ps", tag="q_ps", bufs=2)
    for c in range(2):
        for k in range(2):
            nc.tensor.matmul(kv_ps[:, c, :],
                             lhsT=t1T[:, k, 128 * c:128 * (c + 1)],
                             rhs=wv[:, k, :], start=(k == 0), stop=(k == 1))
    bd = consts.tile([128, 2, 128], BF16, name="bd")
    nc.vector.memset(bd[:], 0.0)
    for c in range(2):
        for r in range(4):
            h = 4 * c + r
            rows = slice(32 * r, 32 * (r + 1))
            nc.scalar.activation(out=bd[rows, c, rows],
                                 in_=kv_ps[rows, c, 32 * h:32 * h + 32], func=AF.Copy)

    psA_cm.__exit__(None, None, None)
    psB_cm = tc.tile_pool(name="psB", bufs=2, space="PSUM")
    psB = psB_cm.__enter__()
    ctx.callback(lambda: psB_cm.__exit__(None, None, None))

    # ---- phase 2: 4-stage software pipeline over 256-token units ----
    # Emission order = per-engine queue order (engines are in-order FIFOs),
    # so stages are emitted skewed, deepest first, to avoid head-of-line
    # blocking of ready work behind a stalled older instruction.
    state = [dict() for _ in range(NT)]

    def stage_a(t):
        """qk + attention, producing attnz = attn/qk in one DVE divide.

        qk is replicated over each head's rows via ksumrep; eps is
        negligible (qk is a sum of ~S positive terms, >> 1e-6)."""
        s = state[t]
        cols = slice(256 * t, 256 * (t + 1))
        qk_ps = psB.tile([128, 2, 256], F32, name="qk_ps", bufs=2)
        for c in range(2):
            nc.tensor.matmul(qk_ps[:, c, :], lhsT=ksumrep[:, c, :],
                             rhs=qe[:, c, cols], start=True, stop=True)
        zrep = work.tile([128, 2, 256], F32, name="zrep", bufs=3)
        nc.vector.reciprocal_approx_fast(out=zrep[:], in_=qk_ps[:])
        attn_ps = psB.tile([128, 2, 256], F32, name="attn_ps", bufs=2)
        for c in range(2):
            nc.tensor.matmul(attn_ps[:, c, :], lhsT=bd[:, c, :],
                             rhs=qe[:, c, cols], start=True, stop=True)
        s["attnz"] = work.tile([128, 2, 256], BF16, name="attnz", bufs=3)
        nc.vector.tensor_mul(s["attnz"][:], attn_ps[:], zrep[:])

    def stage_b1(t):
        """merge + LN1 stats; msg copied off PSUM so it frees early."""
        s = state[t]
        msg_ps = psB.tile([128, 2, 256], F32, name="msg_ps", bufs=2)
        for j in range(2):
            for c in range(2):
                nc.tensor.matmul(msg_ps[:, j, :],
                                 lhsT=s["attnz"][:, c, 128 * j:128 * (j + 1)],
                                 rhs=wm[:, c, :], start=(c == 0), stop=(c == 1))
        del s["attnz"]
        s["msgc"] = work.tile([128, 2, 256], BF16, name="msgc", bufs=3)
        nc.scalar.activation(out=s["msgc"][:], in_=msg_ps[:], func=AF.Copy)
        # stats off the bf16 SBUF copy: 2x DVE mode + cheaper access
        st1 = work.tile([128, 2, 6], F32, name="st1", bufs=3)
        mv1 = s["mv1"] = work.tile([128, 2, 2], F32, name="mv1", bufs=3)
        for j in range(2):
            nc.vector.bn_stats(out=st1[:, j, :], in_=s["msgc"][:, j, :])
            nc.vector.bn_aggr(out=mv1[:, j, :], in_=st1[:, j, :])
        lnv1 = work.tile([128, 2], F32, name="lnv1", bufs=3)
        nc.scalar.activation(out=lnv1[:], in_=mv1[:, :, 1], func=AF.Ln,
                             bias=eps_l[:], scale=1.0)
        s["rstd1"] = work.tile([128, 2], F32, name="rstd1", bufs=3)
        nc.scalar.activation(out=s["rstd1"][:], in_=lnv1[:], func=AF.Exp,
                             scale=-0.5)

    def stage_b2(t):
        """LN1 apply (bf16 SBUF, DVE 2x mode) + transpose to feature-major."""
        s = state[t]
        lnm = work.tile([128, 2, 256], BF16, name="lnm", bufs=3)
        for j in range(2):
            nc.vector.tensor_scalar(
                out=lnm[:, j, :], in0=s["msgc"][:, j, :],
                scalar1=s["mv1"][:, j, 0:1], scalar2=s["rstd1"][:, j:j + 1],
                op0=ALU.subtract, op1=ALU.mult)
        del s["msgc"], s["mv1"], s["rstd1"]
        lnT_ps = psB.tile([128, 2, 256], BF16, name="lnT_ps", tag="attn_ps",
                          bufs=2)
        for j in range(2):
            for c in range(2):
                nc.tensor.transpose(out=lnT_ps[:, c, 128 * j:128 * (j + 1)],
                                    in_=lnm[:, j, 128 * c:128 * (c + 1)],
                                    identity=ident[:])
        s["lnmT"] = work.tile([128, 2, 256], BF16, name="lnmT", bufs=3)
        nc.vector.tensor_copy(out=s["lnmT"][:], in_=lnT_ps[:])

    def stage_c(t):
        """MLP1 + relu(+bias), producing h_sb."""
        s = state[t]
        cols = slice(256 * t, 256 * (t + 1))
        s["h_sb"] = work.tile([128, 4, 256], BF16, name="h_sb", bufs=3)
        for half in range(2):
            h_ps = psB.tile([128, 2, 256], F32, name="h_ps", bufs=2)
            for mi in range(2):
                m = 2 * half + mi
                for k in range(4):
                    rhs = xf[:, k, cols] if k < 2 else s["lnmT"][:, k - 2, :]
                    nc.tensor.matmul(h_ps[:, mi, :], lhsT=w1[:, k, m, :], rhs=rhs,
                                     start=(k == 0), stop=(k == 3))
            if b1p_is_zero:
                nc.scalar.activation(
                    out=s["h_sb"][:, 2 * half:2 * half + 2, :], in_=h_ps[:],
                    func=AF.Relu, scale=1.0)
            else:
                for mi in range(2):
                    m = 2 * half + mi
                    nc.scalar.activation(
                        out=s["h_sb"][:, m, :], in_=h_ps[:, mi, :],
                        func=AF.Relu, bias=b1p[:, m:m + 1], scale=1.0)
        del s["lnmT"]

    def stage_d(t):
        """MLP2 + LN2 + store (residual + b2 added on host)."""
        s = state[t]
        msg2_ps = psB.tile([128, 2, 256], F32, name="msg2_ps", tag="qk_ps",
                           bufs=2)
        for j in range(2):
            for k in range(4):
                nc.tensor.matmul(msg2_ps[:, j, :],
                                 lhsT=s["h_sb"][:, k, 128 * j:128 * (j + 1)],
                                 rhs=w2[:, k, :], start=(k == 0), stop=(k == 3))
        del s["h_sb"]
        st2 = work.tile([128, 2, 6], F32, name="st2", bufs=3)
        mv2 = work.tile([128, 2, 2], F32, name="mv2", bufs=3)
        for j in range(2):
            nc.vector.bn_stats(out=st2[:, j, :], in_=msg2_ps[:, j, :])
            nc.vector.bn_aggr(out=mv2[:, j, :], in_=st2[:, j, :])
        lnv2 = work.tile([128, 2], F32, name="lnv2", bufs=3)
        nc.scalar.activation(out=lnv2[:], in_=mv2[:, :, 1], func=AF.Ln,
                             bias=eps_l[:], scale=1.0)
        rstd2 = work.tile([128, 2], F32, name="rstd2", bufs=3)
        nc.scalar.activation(out=rstd2[:], in_=lnv2[:], func=AF.Exp, scale=-0.5)
        res_sb = work.tile([128, 2, 256], F32, name="res_sb", bufs=3)
        if g2_is_one:
            mb2 = work.tile([128, 2], F32, name="mb2", bufs=3)
            nc.vector.scalar_tensor_tensor(
                out=mb2[:], in0=mv2[:, :, 0], scalar=-1.0, in1=rstd2[:],
                op0=ALU.mult, op1=ALU.mult)
            for j in range(2):
                nc.scalar.activation(out=res_sb[:, j, :], in_=msg2_ps[:, j, :],
                                     func=AF.Identity, bias=mb2[:, j:j + 1],
                                     scale=rstd2[:, j:j + 1])
        else:
            for j in range(2):
                g2r = work.tile([128, 256], F32, name="g2r")
                nc.vector.tensor_scalar_mul(g2r[:], g2rep[:], rstd2[:, j:j + 1])
                nc.vector.scalar_tensor_tensor(
                    out=res_sb[:, j, :], in0=msg2_ps[:, j, :],
                    scalar=mv2[:, j, 0:1], in1=g2r[:],
                    op0=ALU.subtract, op1=ALU.mult)
        nc.sync.dma_start(out=res_d[:, 512 * t:512 * (t + 1)], in_=res_sb[:])

    stages = [stage_d, stage_c, stage_b2, stage_b1, stage_a]  # deepest first
    for step in range(NT + 4):
        for depth, fn in zip((4, 3, 2, 1, 0), stages):
            u = step - depth
            if 0 <= u < NT:
                fn(u)


def _prep_host(inputs, TBLK, SBLK):
    """Shared host-side prep. Returns (const_map, per-core fn)."""
    bf = ml_dtypes.bfloat16
    Wq, Wk, Wv = inputs["Wq"], inputs["Wk"], inputs["Wv"]
    Wm, W1, W2 = inputs["Wmerge"], inputs["Wmlp1"], inputs["Wmlp2"]
    g1, b1 = inputs["ln1_g"], inputs["ln1_b"]
    g2 = inputs["ln2_g"]
    # fold ln1 gamma/beta into W1: h = relu(cat[x, g1*n + b1] @ W1.T)
    W1s = W1.copy()
    W1s[:, 256:] = W1[:, 256:] * g1[None, :]
    b1p = (W1[:, 256:] @ b1).astype(np.float32)          # [512]
    const = {
        "wq_t": np.ascontiguousarray(
            Wq.T.reshape(2, 128, 2, 128).transpose(1, 0, 2, 3))
            .astype(ml_dtypes.float8_e4m3fn),
        "wk_r": np.ascontiguousarray(Wk.T.reshape(2, 128, 256)
                                     .transpose(1, 0, 2)).astype(bf),
        "wv_r": np.ascontiguousarray(Wv.T.reshape(2, 128, 256)
                                     .transpose(1, 0, 2)).astype(bf),
        "wm_r": np.ascontiguousarray(Wm.T.reshape(2, 128, 256)
                                     .transpose(1, 0, 2)).astype(bf),
        "w1_t": np.ascontiguousarray(
            W1s.T.reshape(4, 128, 4, 128).transpose(1, 0, 2, 3)).astype(bf),
        "w2_r": np.ascontiguousarray(W2.T.reshape(4, 128, 256)
                                     .transpose(1, 0, 2)).astype(bf),
        "b1p": np.ascontiguousarray(b1p.reshape(4, 128).T).astype(np.float32),
        "g2rep": np.broadcast_to(g2.astype(np.float32), (128, 256)).copy(),
        "ident": np.eye(128, dtype=bf),
        "maskbd": np.kron(np.eye(4), np.ones((32, 32))).astype(bf),
    }

    def blocks(a, nblk):  # token-major [T, C] -> feature-major [128, 2, T] bf16
        del nblk
        T = a.shape[0]
        return np.ascontiguousarray(
            a.T.reshape(2, 128, T).transpose(1, 0, 2)).astype(bf)

    def tok_blocks(a):  # [S, C] -> token-major [128, (S//128)*264] bf16 + ones col
        S = a.shape[0]
        out = np.ones((128, S // 128, 264), dtype=bf)
        out[:, :, :256] = a.reshape(S // 128, 128, -1).transpose(1, 0, 2).astype(bf)
        return out.reshape(128, -1)

    return const, blocks, tok_blocks


TRACE = False        # set by test harness for NTFF profiling
LAST_RESULT = None   # BassKernelResults of the last kernel() call


def build_nc(g2_is_one=True, TBLK=32, SBLK=64, b1p_is_zero=True):
    """Trace + compile the per-core program; returns the compiled Bacc."""
    nc = bacc.Bacc("TRN2", target_bir_lowering=False, debug=False, num_devices=8)
    with tile.TileContext(nc) as tc:
        nc.tc = tc
        build_kernel(nc, TBLK, SBLK, g2_is_one, b1p_is_zero)
        nc._pool_ctx.close()
    # Pre-place the one activation table that covers every func used
    # (ln/exp/copy/relu); otherwise the table-load pass alternates between
    # the exp and ln tables on every LayerNorm (47 loads, ~60us).
    ld = mybir.InstLoadActFuncSet(name=nc.get_next_instruction_name(),
                                  ins=[], outs=[])
    ld.act_func_set_id = ACT_SET_LN_EXP
    ld.engine = mybir.EngineType.Activation
    nc.register_instruction(ld)
    nc.main_func.blocks[0].instructions.insert(0, ld)
    nc.compile()
    return nc


def kernel(**inputs):
    global LAST_RESULT
    TBLK, SBLK = 32, 64
    N, L, C = inputs["x"].shape
    x = np.asarray(inputs["x"], np.float32)
    source = np.asarray(inputs["source"], np.float32)
    const, blocks, tok_blocks = _prep_host(inputs, TBLK, SBLK)
    g2_is_one = bool(np.allclose(np.asarray(inputs["ln2_g"], np.float32), 1.0))
    b1p_z = bool(np.allclose(
        inputs["Wmlp1"].astype(np.float32)[:, 256:]
        @ np.asarray(inputs["ln1_b"], np.float32), 0.0))
    nc = build_nc(g2_is_one, TBLK, SBLK, b1p_z)

    in_maps = []
    stoks = [tok_blocks(source[n]) for n in range(N)]
    for c in range(8):
        n, half = c // 2, c % 2
        xs = x[n, 4096 * half:4096 * (half + 1)]
        xb = blocks(xs, TBLK)
        in_maps.append({**const,
                        "x_pre": xb,
                        "x_q": xb.astype(ml_dtypes.float8_e4m3fn),
                        "s_pre": blocks(source[n], SBLK),
                        "s_tok": stoks[n]})
    LAST_RESULT = run_bass_kernel_spmd(nc, in_maps, core_ids=list(range(8)),
                                       trace=TRACE)
    res = LAST_RESULT.results

    out = np.empty((N, L, C), np.float32)
    b2 = np.asarray(inputs["ln2_b"], np.float32)
    for c in range(8):
        n, half = c // 2, c % 2
        r = res[c]["res"].reshape(128, TBLK, C).transpose(1, 0, 2).reshape(4096, C)
        out[n, 4096 * half:4096 * (half + 1)] = (
            x[n, 4096 * half:4096 * (half + 1)] + b2[None, :] + r)
    return out
